# revision 8
# baseline (speedup 1.0000x reference)
"""Grouped GEMM (MoE block-diagonal) on 8 Trainium2 NeuronCores.

Problem: x [262144, 256] bf16, w [1024, 256] bf16 (G=8 experts of [128, 256]).
Rows g*32768:(g+1)*32768 of x belong to expert g.
Output [262144, 1024] bf16, block-diagonal: out[rows_g, g*128:(g+1)*128] = x_g @ w_g^T.

Strategy (expert-parallel):
  - Core g gets expert g: x_g [32768, 256] and w_g [128, 256].
  - Host pre-transposes both operands so the contraction dim K lands on SBUF
    partitions (PE matmul contracts over the partition dim) — no on-device
    transpose needed.
  - Device computes yT_g [128, 32768] = w_g @ x_g^T with lhsT = w_g^T
    (stationary) and rhs = x_g^T (moving, 512-token PSUM tiles), so every
    DMA (in and out) moves multi-KB contiguous runs per partition.
  - Host transposes yT_g back and scatters into the zero-filled
    block-diagonal output (the zero blocks never touch the device).
"""

import sys

for _p in ("/opt/trn_rl_repo", "/root/.axon_site/_ro/trn_rl_repo"):
    if _p not in sys.path:
        sys.path.insert(0, _p)

import numpy as np

G = 8          # experts == cores
K = 256        # contraction dim
N = 128        # output dim per expert
M = 262144     # total tokens
MPC = M // G   # tokens per core = 32768

MT = 4096      # tokens per outer tile (SBUF staging)
PT = 512       # tokens per PSUM tile (max matmul free dim)
KP = 128       # partition chunk of K


def _split_multi_waits(nc, mybir):
    """This walrus build rejects any instruction carrying more than one sync
    wait ("Too many sync wait commands", setupSyncWait). Hoist all but one
    wait of each offender onto fresh single-wait EventSemaphore instructions
    placed just before it on the same engine queue — semantically identical
    (sequencer-level blocking, monotonic sem conditions)."""
    for fn in nc.m.functions:
        for blk in fn.blocks:
            new_insts = []
            for inst in blk.instructions:
                si = getattr(inst, "sync_info", None)
                waits = list(si.on_wait) if si is not None and si.on_wait else []
                if len(waits) > 1:
                    for w in waits[:-1]:
                        name = nc.get_next_instruction_name()
                        ev = mybir.InstEventSemaphore(
                            name=name,
                            engine=inst.engine,
                            ins=[],
                            outs=[],
                            sync_info=mybir.SyncInfo(on_wait=[w], on_update=[]),
                        )
                        nc.inst_map[name] = ev
                        new_insts.append(ev)
                    si.on_wait = waits[-1:]
                new_insts.append(inst)
            blk.instructions = new_insts


def _build_bass():
    import concourse.bass as bass
    import concourse.mybir as mybir
    import concourse.tile as tile

    bf16 = mybir.dt.bfloat16
    f32 = mybir.dt.float32

    nc = bass.Bass()
    xT = nc.declare_dram_parameter("xT", [K, MPC], bf16, isOutput=False)
    wT = nc.declare_dram_parameter("wT", [K, N], bf16, isOutput=False)
    yT = nc.declare_dram_parameter("yT", [N, MPC], bf16, isOutput=True)

    with tile.TileContext(nc) as tc:
        with (
            tc.tile_pool(name="w", bufs=1) as wpool,
            tc.tile_pool(name="x", bufs=3) as xpool,
            tc.tile_pool(name="y", bufs=3) as ypool,
            tc.tile_pool(name="ps", bufs=7, space=bass.MemorySpace.PSUM) as pspool,
            tc.tile_pool(name="dmm", bufs=1, space=bass.MemorySpace.PSUM) as dmpool,
        ):
            w_t = wpool.tile([KP, 2, N], bf16)
            nc.gpsimd.dma_start(
                w_t[:], wT[:, :].rearrange("(two p) n -> p two n", two=2)
            )

            for mo in range(0, MPC, MT):
                x_t = xpool.tile([KP, 2, MT], bf16)
                nc.gpsimd.dma_start(
                    x_t[:],
                    xT[:, mo : mo + MT].rearrange("(two p) m -> p two m", two=2),
                )

                # 1x1 dummy matmul reading the fresh x tile: absorbs the
                # x-DMA wait on the PE queue so real matmuls carry at most
                # one sync wait (the TRN2 MM struct supports only one).
                dmm = dmpool.tile([1, 1], f32)
                nc.tensor.matmul(
                    dmm[:], x_t[0:1, 0, 0:1], x_t[0:1, 0, 0:1], start=True, stop=True
                )

                y_t = ypool.tile([N, MT], bf16)
                for ms in range(0, MT, PT):
                    ps = pspool.tile([N, PT], f32)
                    nc.tensor.matmul(
                        ps[:],
                        w_t[:, 0, :],
                        x_t[:, 0, ms : ms + PT],
                        start=True,
                        stop=False,
                    )
                    nc.tensor.matmul(
                        ps[:],
                        w_t[:, 1, :],
                        x_t[:, 1, ms : ms + PT],
                        start=False,
                        stop=True,
                    )
                    nc.vector.tensor_copy(y_t[:, ms : ms + PT], ps[:])

                nc.gpsimd.dma_start(yT[:, mo : mo + MT], y_t[:])

    _split_multi_waits(nc, mybir)
    return nc


_NC_CACHE = None


def _get_nc():
    global _NC_CACHE
    if _NC_CACHE is None:
        _NC_CACHE = _build_bass()
    return _NC_CACHE


def _run(in_maps, **kwargs):
    from concourse.bass_utils import run_bass_kernel_spmd

    return run_bass_kernel_spmd(_get_nc(), in_maps, list(range(G)), **kwargs)


def make_in_maps(x, w):
    x = np.asarray(x)
    w = np.asarray(w)
    in_maps = []
    for g in range(G):
        xg = x[g * MPC : (g + 1) * MPC, :]
        wg = w[g * N : (g + 1) * N, :]
        in_maps.append(
            {
                "xT": np.ascontiguousarray(xg.T),
                "wT": np.ascontiguousarray(wg.T),
            }
        )
    return in_maps


def assemble(results, dtype):
    out = np.zeros((M, G * N), dtype=dtype)
    for g in range(G):
        yTg = np.asarray(results[g]["yT"])
        out[g * MPC : (g + 1) * MPC, g * N : (g + 1) * N] = yTg.T
    return out


def kernel(x, w):
    x = np.asarray(x)
    w = np.asarray(w)
    res = _run(make_in_maps(x, w))
    return assemble(res.results, x.dtype)


# revision 10
# speedup vs baseline: 1.1627x; 1.1627x over previous
"""Grouped GEMM (MoE block-diagonal) on 8 Trainium2 NeuronCores.

Problem: x [262144, 256] bf16, w [1024, 256] bf16 (G=8 experts of [128, 256]).
Rows g*32768:(g+1)*32768 of x belong to expert g.
Output [262144, 1024] bf16, block-diagonal: out[rows_g, g*128:(g+1)*128] = x_g @ w_g^T.

Strategy (expert-parallel):
  - Core g gets expert g: x_g [32768, 256] and w_g [128, 256].
  - Host pre-transposes both operands so the contraction dim K lands on SBUF
    partitions (PE matmul contracts over the partition dim) — no on-device
    transpose needed.
  - Device computes yT_g [128, 32768] = w_g @ x_g^T with lhsT = w_g^T
    (stationary) and rhs = x_g^T (moving, 512-token PSUM tiles), so every
    DMA (in and out) moves multi-KB contiguous runs per partition.
  - Host transposes yT_g back and scatters into the zero-filled
    block-diagonal output (the zero blocks never touch the device).
"""

import sys

for _p in ("/opt/trn_rl_repo", "/root/.axon_site/_ro/trn_rl_repo"):
    if _p not in sys.path:
        sys.path.insert(0, _p)

import numpy as np

G = 8          # experts == cores
K = 256        # contraction dim
N = 128        # output dim per expert
M = 262144     # total tokens
MPC = M // G   # tokens per core = 32768

MT = 4096      # tokens per outer tile (SBUF staging)
PT = 512       # tokens per PSUM tile (max matmul free dim)
KP = 128       # partition chunk of K


def _split_multi_waits(nc, mybir):
    """This walrus build rejects any instruction carrying more than one sync
    wait ("Too many sync wait commands", setupSyncWait). Hoist all but one
    wait of each offender onto fresh single-wait EventSemaphore instructions
    placed just before it on the same engine queue — semantically identical
    (sequencer-level blocking, monotonic sem conditions)."""
    for fn in nc.m.functions:
        for blk in fn.blocks:
            new_insts = []
            for inst in blk.instructions:
                si = getattr(inst, "sync_info", None)
                waits = list(si.on_wait) if si is not None and si.on_wait else []
                if len(waits) > 1:
                    for w in waits[:-1]:
                        name = nc.get_next_instruction_name()
                        ev = mybir.InstEventSemaphore(
                            name=name,
                            engine=inst.engine,
                            ins=[],
                            outs=[],
                            sync_info=mybir.SyncInfo(on_wait=[w], on_update=[]),
                        )
                        nc.inst_map[name] = ev
                        new_insts.append(ev)
                    si.on_wait = waits[-1:]
                new_insts.append(inst)
            blk.instructions = new_insts


def _build_bass():
    import concourse.bass as bass
    import concourse.mybir as mybir
    import concourse.tile as tile

    bf16 = mybir.dt.bfloat16
    f32 = mybir.dt.float32

    nc = bass.Bass()
    xT = nc.declare_dram_parameter("xT", [K, MPC], bf16, isOutput=False)
    wT = nc.declare_dram_parameter("wT", [K, N], bf16, isOutput=False)
    yT = nc.declare_dram_parameter("yT", [N, MPC], bf16, isOutput=True)

    with tile.TileContext(nc) as tc:
        with (
            tc.tile_pool(name="w", bufs=1) as wpool,
            tc.tile_pool(name="x", bufs=4) as xpool,
            tc.tile_pool(name="y", bufs=3) as ypool,
            tc.tile_pool(name="ps", bufs=7, space=bass.MemorySpace.PSUM) as pspool,
            tc.tile_pool(name="dmm", bufs=1, space=bass.MemorySpace.PSUM) as dmpool,
        ):
            w_t = wpool.tile([KP, 2, N], bf16)
            nc.sync.dma_start(
                w_t[:], wT[:, :].rearrange("(two p) n -> p two n", two=2)
            )

            for mo in range(0, MPC, MT):
                x_t = xpool.tile([KP, 2, MT], bf16)
                nc.sync.dma_start(
                    x_t[:],
                    xT[:, mo : mo + MT].rearrange("(two p) m -> p two m", two=2),
                )

                # 1x1 dummy matmul reading the fresh x tile: absorbs the
                # x-DMA wait on the PE queue so real matmuls carry at most
                # one sync wait (the TRN2 MM struct supports only one).
                dmm = dmpool.tile([1, 1], f32)
                nc.tensor.matmul(
                    dmm[:], x_t[0:1, 0, 0:1], x_t[0:1, 0, 0:1], start=True, stop=True
                )

                y_t = ypool.tile([N, MT], bf16)
                for i, ms in enumerate(range(0, MT, PT)):
                    ps = pspool.tile([N, PT], f32)
                    nc.tensor.matmul(
                        ps[:],
                        w_t[:, 0, :],
                        x_t[:, 0, ms : ms + PT],
                        start=True,
                        stop=False,
                    )
                    nc.tensor.matmul(
                        ps[:],
                        w_t[:, 1, :],
                        x_t[:, 1, ms : ms + PT],
                        start=False,
                        stop=True,
                    )
                    if i % 3 == 2:
                        nc.scalar.copy(y_t[:, ms : ms + PT], ps[:])
                    else:
                        nc.vector.tensor_copy(y_t[:, ms : ms + PT], ps[:])

                nc.scalar.dma_start(yT[:, mo : mo + MT], y_t[:])

    _split_multi_waits(nc, mybir)
    return nc


_NC_CACHE = None


def _get_nc():
    global _NC_CACHE
    if _NC_CACHE is None:
        _NC_CACHE = _build_bass()
    return _NC_CACHE


def _run(in_maps, **kwargs):
    from concourse.bass_utils import run_bass_kernel_spmd

    return run_bass_kernel_spmd(_get_nc(), in_maps, list(range(G)), **kwargs)


def make_in_maps(x, w):
    x = np.asarray(x)
    w = np.asarray(w)
    in_maps = []
    for g in range(G):
        xg = x[g * MPC : (g + 1) * MPC, :]
        wg = w[g * N : (g + 1) * N, :]
        in_maps.append(
            {
                "xT": np.ascontiguousarray(xg.T),
                "wT": np.ascontiguousarray(wg.T),
            }
        )
    return in_maps


def assemble(results, dtype):
    out = np.zeros((M, G * N), dtype=dtype)
    for g in range(G):
        yTg = np.asarray(results[g]["yT"])
        out[g * MPC : (g + 1) * MPC, g * N : (g + 1) * N] = yTg.T
    return out


def kernel(x, w):
    x = np.asarray(x)
    w = np.asarray(w)
    res = _run(make_in_maps(x, w))
    return assemble(res.results, x.dtype)
